# revision 11
# baseline (speedup 1.0000x reference)
"""BrainGAT (2x GATv2Conv + residuals + FC) on 8 Trainium2 NeuronCores, v2.

Sharding: nodes partitioned across 8 cores via a load-balanced permutation
(160 bins of 128 slots, 125 real nodes each, in-degree balanced by snake
dealing); edges assigned to the bin owning their destination. Small weights
replicated. Per-layer source features exchanged with one bf16 AllGather.

Edge phase per 2048-edge block: one dma_gather fetches all 16 edge tiles'
source features, a second dma_gather fetches destination-side features, and
all elementwise work runs as whole-block batched bf16 vector ops. Segment
softmax uses the exact no-max rewrite: alpha_e = exp(logit_e) /
(sum exp + exp(m_d)) where m_d is the self-loop logit, so self-loops never
enter the edge lists and the softmax-max pass is skipped (logits are O(6)).
"""
import numpy as np

import concourse.bass as bass
import concourse.bacc as bacc
import concourse.mybir as mybir
import concourse.tile as tile
from concourse.bass_utils import run_bass_kernel_spmd
from concourse.masks import make_identity

f32 = mybir.dt.float32
bf16 = mybir.dt.bfloat16
f32r = mybir.dt.float32r
i16 = mybir.dt.int16
NPBF = mybir.dt.np(bf16)
AF = mybir.ActivationFunctionType
ALU = mybir.AluOpType
PRELU = AF.Prelu  # sim_test overrides (Prelu unimplemented in CoreSim)

NC = 8
HEADS = 4
NEG_SLOPE = 0.2
P = 128
B = 128            # dst-block size (slots per bin)
NB = 20            # blocks per core
NPC = B * NB       # node slots per core (2560; 2500 real)
NREAL = 2500       # real nodes per core
NFULL = NC * NPC   # padded global table rows
IN_CH = 128
HC1 = 256
HC2 = 128
OUT_CH = 64


# ----------------------------------------------------------------------------
# device program
# ----------------------------------------------------------------------------

def build_program(NT, dbg=False):
    EB = NT * P               # edges per block (padded)
    IC = EB // 16             # idx cols per block
    nc = bacc.Bacc("TRN2", target_bir_lowering=False, debug=False)

    def inp(name, shape, dt=f32):
        return nc.dram_tensor(name, shape, dt, kind="ExternalInput")

    xT = inp("xT", [IN_CH, NPC], f32r)
    src_idx = inp("src_idx", [P, NB * IC], i16)
    dstrow = inp("dstrow", [NB, EB], bf16)
    dstslot = inp("dstslot", [P, NB * NT], bf16)
    iota_row = inp("iota_row", [P, P], bf16)
    iota_col = inp("iota_col", [P, 1], bf16)

    Wl1T = inp("Wl1T", [IN_CH, HC1], f32r); Wr1T = inp("Wr1T", [IN_CH, HC1], f32r)
    P1T = inp("P1T", [IN_CH, HC1], f32r)
    bl1_bc = inp("bl1_bc", [P, HC1]); br1_bc = inp("br1_bc", [P, HC1])
    pb1m1_bc = inp("pb1m1_bc", [P, HC1]); bias1_bc = inp("bias1_bc", [P, HC1])
    att1_bc = inp("att1_bc", [P, HC1], bf16)

    Wl2T = inp("Wl2T", [HC1, HC2], bf16); Wr2T = inp("Wr2T", [HC1, HC2], bf16)
    P2T = inp("P2T", [HC1, HC2], bf16)
    bl2_bc = inp("bl2_bc", [P, HC2]); br2_bc = inp("br2_bc", [P, HC2])
    pb2m1_bc = inp("pb2m1_bc", [P, HC2]); bias2_bc = inp("bias2_bc", [P, HC2])
    att2_bc = inp("att2_bc", [P, HC2], bf16)

    WfT = inp("WfT", [HC2, OUT_CH], bf16); bf_bc = inp("bf_bc", [P, OUT_CH])

    xl1_own = nc.dram_tensor("xl1_own", [NPC, HC1], bf16)
    xl1_full = nc.dram_tensor("xl1_full", [NFULL, HC1], bf16, addr_space="Shared")
    xl2_own = nc.dram_tensor("xl2_own", [NPC, HC2], bf16)
    xl2_full = nc.dram_tensor("xl2_full", [NFULL, HC2], bf16, addr_space="Shared")
    out_own = nc.dram_tensor("out_own", [NPC, OUT_CH], f32, kind="ExternalOutput")

    with tile.TileContext(nc) as tc:
        with (
            tc.tile_pool(name="const", bufs=1) as cp,
            tc.tile_pool(name="res", bufs=1) as rp,
            tc.tile_pool(name="work", bufs=2) as wp,
            tc.tile_pool(name="gath", bufs=3) as gp,
        ):
            # ---- constants
            ident = cp.tile([P, P], f32)
            make_identity(nc, ident[:])
            identb = cp.tile([P, P], bf16)
            nc.vector.tensor_copy(out=identb[:], in_=ident[:])
            with tc.tile_pool(name="pwarm", bufs=1, space="PSUM") as pw:
                warm = pw.tile([P, P], f32)
                nc.tensor.transpose(out=warm[:], in_=ident[:], identity=ident[:])

            def load_const(t, shape):
                s = cp.tile(shape, t.dtype, tag=f"c_{t.name}")
                nc.sync.dma_start(out=s[:], in_=t[:, :])
                return s

            iota_sb = load_const(iota_row, [P, P])
            iotac_sb = load_const(iota_col, [P, 1])
            zero1_sb = cp.tile([P, HC1], f32)
            nc.gpsimd.memset(zero1_sb[:], 0.0)
            one1_sb = cp.tile([P, HC1], f32)
            nc.gpsimd.memset(one1_sb[:], 1.0)
            w1 = {k: load_const(v, [IN_CH, HC1])
                  for k, v in (("wl", Wl1T), ("wr", Wr1T), ("p", P1T))}
            c1b = {k: load_const(v, [P, HC1]) for k, v in (
                ("bl", bl1_bc), ("br", br1_bc), ("pbm1", pb1m1_bc),
                ("att", att1_bc), ("bias", bias1_bc))}

            def load_w2(t):
                # [HC1, HC2] DRAM -> [128, 2*HC2] SBUF (chunk c at col c*HC2)
                s = cp.tile([P, 2 * HC2], t.dtype, tag=f"c_{t.name}")
                for c in range(2):
                    nc.sync.dma_start(out=s[:, c * HC2:(c + 1) * HC2],
                                      in_=t[c * P:(c + 1) * P, :])
                return s

            w2 = {k: load_w2(v) for k, v in (("wl", Wl2T), ("wr", Wr2T),
                                             ("p", P2T))}
            c2b = {k: load_const(v, [P, HC2]) for k, v in (
                ("bl", bl2_bc), ("br", br2_bc), ("pbm1", pb2m1_bc),
                ("att", att2_bc), ("bias", bias2_bc))}
            wf_sb = load_const(WfT, [HC2, OUT_CH])
            bf_sb = load_const(bf_bc, [P, OUT_CH])

            xT_sb = rp.tile([IN_CH, NPC], f32r)
            nc.sync.dma_start(out=xT_sb[:], in_=xT[:, :])
            sidx_sb = rp.tile([P, NB * IC], i16)
            nc.sync.dma_start(out=sidx_sb[:], in_=src_idx[:, :])
            dsl_sb = rp.tile([P, NB * NT], bf16)
            nc.sync.dma_start(out=dsl_sb[:], in_=dstslot[:, :])

            # ---- persistent per-layer node tensors
            xl1_sb = rp.tile([P, NB * HC1], bf16)
            xr1_sb = rp.tile([P, NB * HC1], bf16)
            id1_sb = rp.tile([P, NB * HC1], bf16)
            em1_sb = rp.tile([P, NB * HEADS], f32)
            h_sb = rp.tile([P, NB * HC1], bf16)
            hT_sb = rp.tile([P, NB * HC1], bf16)     # h transposed, 2 chunks/blk
            xl2_sb = rp.tile([P, NB * HC2], bf16)
            xr2_sb = rp.tile([P, NB * HC2], bf16)
            id2_sb = rp.tile([P, NB * HC2], bf16)
            em2_sb = rp.tile([P, NB * HEADS], f32)
            h2_sb = rp.tile([P, NB * HC2], bf16)

            # ---- N1a: xl for all blocks, then AllGather
            with tc.tile_pool(name="pn1a", bufs=2, space="PSUM") as ps:
                for b in range(NB):
                    pxl = ps.tile([P, HC1], f32, tag="pxl")
                    nc.tensor.matmul(out=pxl[:], lhsT=xT_sb[:, b * B:(b + 1) * B],
                                     rhs=w1["wl"][:], start=True, stop=True)
                    nc.vector.tensor_add(out=xl1_sb[:, b * HC1:(b + 1) * HC1],
                                         in0=pxl[:], in1=c1b["bl"][:])
            nc.sync.dma_start(
                out=xl1_own.ap().rearrange("(b p) c -> p b c", p=B),
                in_=xl1_sb[:].rearrange("p (b c) -> p b c", b=NB))
            nc.gpsimd.collective_compute(
                "AllGather", ALU.bypass,
                replica_groups=[list(range(NC))],
                ins=[xl1_own.ap().opt()], outs=[xl1_full.ap().opt()])

            # ---- N1c: xr, id, m (overlaps the AllGather)
            with tc.tile_pool(name="pn1c", bufs=2, space="PSUM") as ps:
                for b in range(NB):
                    pxr = ps.tile([P, HC1], f32, tag="pxr")
                    pid = ps.tile([P, HC1], f32, tag="pid")
                    lhsT = xT_sb[:, b * B:(b + 1) * B]
                    nc.tensor.matmul(out=pxr[:], lhsT=lhsT, rhs=w1["wr"][:],
                                     start=True, stop=True)
                    nc.tensor.matmul(out=pid[:], lhsT=lhsT, rhs=w1["p"][:],
                                     start=True, stop=True)
                    xr_t = xr1_sb[:, b * HC1:(b + 1) * HC1]
                    nc.vector.tensor_add(out=xr_t, in0=pxr[:], in1=c1b["br"][:])
                    nc.vector.tensor_add(out=id1_sb[:, b * HC1:(b + 1) * HC1],
                                         in0=pid[:], in1=c1b["pbm1"][:])
                    t0 = wp.tile([P, HC1], bf16, tag="t0")
                    nc.vector.tensor_add(out=t0[:],
                                         in0=xl1_sb[:, b * HC1:(b + 1) * HC1],
                                         in1=xr_t)
                    nc.scalar.activation(t0[:], t0[:], PRELU, alpha=NEG_SLOPE)
                    nc.vector.tensor_mul(out=t0[:], in0=t0[:], in1=c1b["att"][:])
                    nc.vector.reduce_sum(
                        out=em1_sb[:, b * HEADS:(b + 1) * HEADS],
                        in_=t0[:].rearrange("p (h c) -> p h c", h=HEADS),
                        axis=mybir.AxisListType.X)
            nc.scalar.activation(em1_sb[:], em1_sb[:], AF.Exp)

            # ---- generic edge phase
            # tt = xl_src + xr_dst built on the PE: per tile, ohT @ xr_block
            # accumulated with I @ xlg in PSUM; Prelu reads PSUM on Scalar.
            def edge_phase(b, full_tab, xr_sb, xl_sb, id_sb, em_sb, hout_sb,
                           hc, consts, psa, pst):
                ic0 = b * IC
                hf = NT // 2          # dma_gather wedges above 1024 idxs
                eh, ich = hf * P, IC // 2
                xlg = gp.tile([P, NT, hc], bf16, tag="xlg")
                for i in range(2):
                    nc.gpsimd.dma_gather(
                        xlg[:, i * hf:(i + 1) * hf, :], full_tab[:, :],
                        sidx_sb[:, ic0 + i * ich:ic0 + (i + 1) * ich],
                        eh, eh, hc)
                dr = gp.tile([P, EB], bf16, tag="dr")
                nc.sync.dma_start(
                    out=dr[:], in_=dstrow.ap()[b:b + 1, :].partition_broadcast(P))
                ohT = wp.tile([P, NT, P], bf16, tag="ohT")
                nc.vector.tensor_tensor(
                    out=ohT[:].rearrange("p t q -> p (t q)"),
                    in0=iotac_sb[:].to_broadcast([P, EB]),
                    in1=dr[:], op=ALU.is_equal)
                oh = wp.tile([P, NT, P], bf16, tag="oh")
                nc.vector.tensor_tensor(
                    out=oh[:],
                    in0=dsl_sb[:, b * NT:(b + 1) * NT].unsqueeze(2)
                        .to_broadcast([P, NT, P]),
                    in1=iota_sb[:].unsqueeze(1).to_broadcast([P, NT, P]),
                    op=ALU.is_equal)
                xrb = xr_sb[:, b * hc:(b + 1) * hc]
                tb = wp.tile([P, NT, hc], bf16, tag="tb")
                for j in range(NT // 2):
                    ttp = pst.tile([P, 2, hc], f32, tag="ttp")
                    for i in range(2):
                        k = 2 * j + i
                        nc.tensor.matmul(out=ttp[:, i, :], lhsT=ohT[:, k, :],
                                         rhs=xrb, start=True, stop=False)
                        nc.tensor.matmul(out=ttp[:, i, :], lhsT=identb[:],
                                         rhs=xlg[:, k, :], start=False, stop=True)
                    nc.scalar.activation(
                        tb[:, 2 * j:2 * j + 2, :].rearrange("p t e -> p (t e)"),
                        ttp[:].rearrange("p t e -> p (t e)"),
                        PRELU, alpha=NEG_SLOPE)
                nc.vector.tensor_mul(
                    out=tb[:], in0=tb[:],
                    in1=consts["att"][:].unsqueeze(1).to_broadcast([P, NT, hc]))
                lgf = wp.tile([P, NT * HEADS], f32, tag="lgf")
                nc.vector.reduce_sum(
                    out=lgf[:].rearrange("p (t h) -> p t h", t=NT),
                    in_=tb[:].rearrange("p t (h c) -> p t h c", h=HEADS),
                    axis=mybir.AxisListType.X)
                pexp = wp.tile([P, NT * HEADS], bf16, tag="pexp")
                nc.scalar.activation(pexp[:], lgf[:], AF.Exp)
                wptn = wp.tile([P, NT, hc], bf16, tag="wptn")
                nc.vector.tensor_tensor(
                    out=wptn[:].rearrange("p t (h c) -> p t h c", h=HEADS),
                    in0=xlg[:].rearrange("p t (h c) -> p t h c", h=HEADS),
                    in1=pexp[:].rearrange("p (t h) -> p t h", t=NT).unsqueeze(3)
                        .to_broadcast([P, NT, HEADS, hc // HEADS]),
                    op=ALU.mult)
                seg = psa.tile([P, hc], f32, tag="seg")
                segE = psa.tile([P, HEADS], f32, tag="segE")
                pex3 = pexp[:].rearrange("p (t h) -> p t h", t=NT)
                for k in range(NT):
                    nc.tensor.matmul(out=seg[:], lhsT=oh[:, k, :],
                                     rhs=wptn[:, k, :],
                                     start=(k == 0), stop=(k == NT - 1))
                for k in range(NT):
                    nc.tensor.matmul(out=segE[:], lhsT=oh[:, k, :],
                                     rhs=pex3[:, k, :],
                                     start=(k == 0), stop=(k == NT - 1))
                # finalize block b
                e4 = em_sb[:, b * HEADS:(b + 1) * HEADS]
                den = wp.tile([P, HEADS], f32, tag="den")
                nc.vector.tensor_add(out=den[:], in0=segE[:], in1=e4)
                rec = wp.tile([P, HEADS], f32, tag="rec")
                nc.vector.reciprocal(out=rec[:], in_=den[:])
                num = wp.tile([P, hc], f32, tag="num")
                nc.vector.tensor_tensor(
                    out=num[:].rearrange("p (h c) -> p h c", h=HEADS),
                    in0=xl_sb[:, b * hc:(b + 1) * hc]
                        .rearrange("p (h c) -> p h c", h=HEADS),
                    in1=e4.unsqueeze(2).to_broadcast([P, HEADS, hc // HEADS]),
                    op=ALU.mult)
                nc.vector.tensor_add(out=num[:], in0=num[:], in1=seg[:])
                nc.vector.tensor_tensor(
                    out=num[:].rearrange("p (h c) -> p h c", h=HEADS),
                    in0=num[:].rearrange("p (h c) -> p h c", h=HEADS),
                    in1=rec[:].unsqueeze(2).to_broadcast([P, HEADS, hc // HEADS]),
                    op=ALU.mult)
                nc.vector.tensor_add(out=num[:], in0=num[:], in1=consts["bias"][:])
                eu = wp.tile([P, hc], f32, tag="eu")
                nc.scalar.activation(eu[:], num[:], AF.Exp)
                nc.vector.tensor_tensor(out=eu[:], in0=eu[:],
                                        in1=one1_sb[:, :hc], op=ALU.min)
                nc.vector.tensor_tensor(out=num[:], in0=num[:],
                                        in1=zero1_sb[:, :hc], op=ALU.max)
                nc.vector.tensor_add(out=num[:], in0=num[:], in1=eu[:])
                nc.vector.tensor_add(out=hout_sb[:, b * hc:(b + 1) * hc],
                                     in0=num[:], in1=id_sb[:, b * hc:(b + 1) * hc])

            with (
                tc.tile_pool(name="pe1", bufs=2, space="PSUM") as psa,
                tc.tile_pool(name="pt1", bufs=4, space="PSUM") as pst,
            ):
                for b in range(NB):
                    edge_phase(b, xl1_full, xr1_sb, xl1_sb, id1_sb, em1_sb,
                               h_sb, HC1, c1b, psa, pst)

            # ---- N2a: transpose h; N2b: xl2 + AllGather; N2c: xr2/id2/m2
            with tc.tile_pool(name="pn2a", bufs=2, space="PSUM") as ps:
                for b in range(NB):
                    for c in range(2):
                        tps = ps.tile([P, P], bf16, tag="hT_ps")
                        nc.tensor.transpose(
                            out=tps[:],
                            in_=h_sb[:, b * HC1 + c * P:b * HC1 + (c + 1) * P],
                            identity=identb[:])
                        nc.scalar.copy(
                            out=hT_sb[:, b * HC1 + c * P:b * HC1 + (c + 1) * P],
                            in_=tps[:])
            with tc.tile_pool(name="pn2b", bufs=2, space="PSUM") as ps:
                for b in range(NB):
                    pxl = ps.tile([P, HC2], f32, tag="p2xl")
                    for c in range(2):
                        nc.tensor.matmul(
                            out=pxl[:],
                            lhsT=hT_sb[:, b * HC1 + c * P:b * HC1 + (c + 1) * P],
                            rhs=w2["wl"][:, c * HC2:(c + 1) * HC2],
                            start=(c == 0), stop=(c == 1))
                    nc.vector.tensor_add(out=xl2_sb[:, b * HC2:(b + 1) * HC2],
                                         in0=pxl[:], in1=c2b["bl"][:])
            nc.sync.dma_start(
                out=xl2_own.ap().rearrange("(b p) c -> p b c", p=B),
                in_=xl2_sb[:].rearrange("p (b c) -> p b c", b=NB))
            nc.gpsimd.collective_compute(
                "AllGather", ALU.bypass,
                replica_groups=[list(range(NC))],
                ins=[xl2_own.ap().opt()], outs=[xl2_full.ap().opt()])
            with tc.tile_pool(name="pn2c", bufs=2, space="PSUM") as ps:
                for b in range(NB):
                    pxr = ps.tile([P, HC2], f32, tag="p2xr")
                    pid = ps.tile([P, HC2], f32, tag="p2id")
                    for c in range(2):
                        lhsT = hT_sb[:, b * HC1 + c * P:b * HC1 + (c + 1) * P]
                        st, sp = (c == 0), (c == 1)
                        cs = slice(c * HC2, (c + 1) * HC2)
                        nc.tensor.matmul(out=pxr[:], lhsT=lhsT,
                                         rhs=w2["wr"][:, cs], start=st, stop=sp)
                        nc.tensor.matmul(out=pid[:], lhsT=lhsT,
                                         rhs=w2["p"][:, cs], start=st, stop=sp)
                    xr_t = xr2_sb[:, b * HC2:(b + 1) * HC2]
                    nc.vector.tensor_add(out=xr_t, in0=pxr[:], in1=c2b["br"][:])
                    nc.vector.tensor_add(out=id2_sb[:, b * HC2:(b + 1) * HC2],
                                         in0=pid[:], in1=c2b["pbm1"][:])
                    t0 = wp.tile([P, HC2], bf16, tag="t02")
                    nc.vector.tensor_add(out=t0[:],
                                         in0=xl2_sb[:, b * HC2:(b + 1) * HC2],
                                         in1=xr_t)
                    nc.scalar.activation(t0[:], t0[:], PRELU, alpha=NEG_SLOPE)
                    nc.vector.tensor_mul(out=t0[:], in0=t0[:], in1=c2b["att"][:])
                    nc.vector.reduce_sum(
                        out=em2_sb[:, b * HEADS:(b + 1) * HEADS],
                        in_=t0[:].rearrange("p (h c) -> p h c", h=HEADS),
                        axis=mybir.AxisListType.X)
            nc.scalar.activation(em2_sb[:], em2_sb[:], AF.Exp)

            # ---- layer-2 edge phase + FC
            with (
                tc.tile_pool(name="pe2", bufs=2, space="PSUM") as psa,
                tc.tile_pool(name="pt2", bufs=4, space="PSUM") as pst,
            ):
                for b in range(NB):
                    edge_phase(b, xl2_full, xr2_sb, xl2_sb, id2_sb, em2_sb,
                               h2_sb, HC2, c2b, psa, pst)
            with tc.tile_pool(name="pfc", bufs=2, space="PSUM") as ps:
                for b in range(NB):
                    tps = ps.tile([P, P], bf16, tag="fc_ps")
                    nc.tensor.transpose(
                        out=tps[:], in_=h2_sb[:, b * HC2:(b + 1) * HC2],
                        identity=identb[:])
                    h2T = wp.tile([P, P], bf16, tag="fcT")
                    nc.scalar.copy(out=h2T[:], in_=tps[:])
                    pf = ps.tile([P, OUT_CH], f32, tag="fc_out")
                    nc.tensor.matmul(out=pf[:], lhsT=h2T[:], rhs=wf_sb[:],
                                     start=True, stop=True)
                    ob = wp.tile([P, OUT_CH], f32, tag="fc_ob")
                    nc.vector.tensor_add(out=ob[:], in0=pf[:], in1=bf_sb[:])
                    nc.sync.dma_start(out=out_own.ap()[b * B:(b + 1) * B, :],
                                      in_=ob[:])
    nc.compile()
    return nc


# ----------------------------------------------------------------------------
# host-side sharding / input prep
# ----------------------------------------------------------------------------

def balanced_assignment(dst):
    """Snake-deal nodes (by in-degree desc) into NC*NB bins of NREAL/NB..."""
    N = NC * NREAL
    nbins = NC * NB
    rounds = N // nbins  # 125
    deg = np.bincount(dst, minlength=N)
    order = np.argsort(-deg, kind="stable")
    bin_of = np.empty(N, np.int32)
    slot_of = np.empty(N, np.int32)
    cols = np.arange(nbins)
    for r in range(rounds):
        nodes = order[r * nbins:(r + 1) * nbins]
        c = cols if r % 2 == 0 else cols[::-1]
        bin_of[nodes] = c
        slot_of[nodes] = r
    return bin_of, slot_of


def wrap_idx(vals, EB):
    """[EB] int -> [128, EB//16] int16 wrapped in 16 partitions, replicated."""
    w = np.zeros((16, EB // 16), np.int16)
    w[np.arange(EB) % 16, np.arange(EB) // 16] = vals.astype(np.int16)
    return np.tile(w, (8, 1))


def prep_inputs(x, edge_index, weights):
    src = np.asarray(edge_index[0], dtype=np.int64)
    dst = np.asarray(edge_index[1], dtype=np.int64)
    E = src.shape[0]

    bin_of, slot_of = balanced_assignment(dst)
    core_of = bin_of // NB
    block_of = bin_of % NB
    gid = core_of * NPC + block_of * B + slot_of  # row in full tables

    ebin = bin_of[dst]
    counts = np.bincount(ebin, minlength=NC * NB)
    NT = max(1, int(np.ceil(counts.max() / P)))
    EB = NT * P
    IC = EB // 16

    eorder = np.argsort(ebin, kind="stable")
    offs = np.zeros(NC * NB + 1, np.int64)
    np.cumsum(counts, out=offs[1:])

    src_gid = gid[src]
    dst_slot = slot_of[dst]

    in_maps = []
    for c in range(NC):
        sarr = np.zeros((P, NB * IC), np.int16)
        drow = np.full((NB, EB), -1.0, np.float32)
        slarr = np.full((P, NB * NT), -1.0, np.float32)
        for b in range(NB):
            bi = c * NB + b
            eb = eorder[offs[bi]:offs[bi + 1]]
            n = len(eb)
            sv = np.zeros(EB, np.int64); sv[:n] = src_gid[eb]
            sarr[:, b * IC:(b + 1) * IC] = wrap_idx(sv, EB)
            drow[b, :n] = dst_slot[eb]
            j = np.arange(n)
            slarr[j % P, b * NT + j // P] = dst_slot[eb]
        in_maps.append({"src_idx": sarr, "dstrow": drow.astype(NPBF),
                        "dstslot": slarr.astype(NPBF)})

    def bc(v):
        return np.tile(np.asarray(v, np.float32)[None, :], (P, 1))

    consts = {
        "iota_row": np.tile(np.arange(P), (P, 1)).astype(NPBF),
        "iota_col": np.arange(P).reshape(P, 1).astype(NPBF),
        "Wl1T": np.ascontiguousarray(weights["Wl1"].T.astype(np.float32)),
        "Wr1T": np.ascontiguousarray(weights["Wr1"].T.astype(np.float32)),
        "P1T": np.ascontiguousarray(weights["P1"].T.astype(np.float32)),
        "bl1_bc": bc(weights["bl1"]), "br1_bc": bc(weights["br1"]),
        "pb1m1_bc": bc(weights["pb1"] - 1.0), "bias1_bc": bc(weights["bias1"]),
        "att1_bc": bc(weights["att1"].reshape(-1)).astype(NPBF),
        "Wl2T": np.ascontiguousarray(weights["Wl2"].T.astype(NPBF)),
        "Wr2T": np.ascontiguousarray(weights["Wr2"].T.astype(NPBF)),
        "P2T": np.ascontiguousarray(weights["P2"].T.astype(NPBF)),
        "bl2_bc": bc(weights["bl2"]), "br2_bc": bc(weights["br2"]),
        "pb2m1_bc": bc(weights["pb2"] - 1.0), "bias2_bc": bc(weights["bias2"]),
        "att2_bc": bc(weights["att2"].reshape(-1)).astype(NPBF),
        "WfT": np.ascontiguousarray(weights["Wf"].T.astype(NPBF)),
        "bf_bc": bc(weights["bf"]),
    }

    # permuted x, transposed: col (block*B+slot) = x[node]
    xf = np.asarray(x, np.float32)
    for c in range(NC):
        xp = np.zeros((NPC, IN_CH), np.float32)
        m = core_of == c
        xp[block_of[m] * B + slot_of[m]] = xf[m]
        im = in_maps[c]
        im["xT"] = np.ascontiguousarray(xp.T)
        im.update(consts)
    return in_maps, NT, (core_of, block_of, slot_of)


_CACHE = {}


def kernel(x, edge_index, Wl1, bl1, Wr1, br1, att1, bias1, P1, pb1,
           Wl2, bl2, Wr2, br2, att2, bias2, P2, pb2, Wf, bf):
    x = np.asarray(x)
    weights = dict(Wl1=np.asarray(Wl1), bl1=np.asarray(bl1),
                   Wr1=np.asarray(Wr1), br1=np.asarray(br1),
                   att1=np.asarray(att1), bias1=np.asarray(bias1),
                   P1=np.asarray(P1), pb1=np.asarray(pb1),
                   Wl2=np.asarray(Wl2), bl2=np.asarray(bl2),
                   Wr2=np.asarray(Wr2), br2=np.asarray(br2),
                   att2=np.asarray(att2), bias2=np.asarray(bias2),
                   P2=np.asarray(P2), pb2=np.asarray(pb2),
                   Wf=np.asarray(Wf), bf=np.asarray(bf))
    assert x.shape[0] == NC * NREAL, "hardcoded for the BrainGAT problem size"
    in_maps, NT, (core_of, block_of, slot_of) = prep_inputs(
        x, np.asarray(edge_index), weights)
    if NT not in _CACHE:
        _CACHE[NT] = build_program(NT)
    nc = _CACHE[NT]
    res = run_bass_kernel_spmd(nc, in_maps, list(range(NC)))
    full = np.concatenate([res.results[c]["out_own"] for c in range(NC)], 0)
    rows = core_of * NPC + block_of * B + slot_of
    return full[rows].astype(np.float32)


# revision 12
# speedup vs baseline: 1.0655x; 1.0655x over previous
"""BrainGAT (2x GATv2Conv + residuals + FC) on 8 Trainium2 NeuronCores, v2.

Sharding: nodes partitioned across 8 cores via a load-balanced permutation
(160 bins of 128 slots, 125 real nodes each, in-degree balanced by snake
dealing); edges assigned to the bin owning their destination. Small weights
replicated. Per-layer source features exchanged with one bf16 AllGather.

Edge phase per 2048-edge block: one dma_gather fetches all 16 edge tiles'
source features, a second dma_gather fetches destination-side features, and
all elementwise work runs as whole-block batched bf16 vector ops. Segment
softmax uses the exact no-max rewrite: alpha_e = exp(logit_e) /
(sum exp + exp(m_d)) where m_d is the self-loop logit, so self-loops never
enter the edge lists and the softmax-max pass is skipped (logits are O(6)).
"""
import numpy as np

import concourse.bass as bass
import concourse.bacc as bacc
import concourse.mybir as mybir
import concourse.tile as tile
from concourse.bass_utils import run_bass_kernel_spmd
from concourse.masks import make_identity

f32 = mybir.dt.float32
bf16 = mybir.dt.bfloat16
f32r = mybir.dt.float32r
i16 = mybir.dt.int16
NPBF = mybir.dt.np(bf16)
AF = mybir.ActivationFunctionType
ALU = mybir.AluOpType
PRELU = AF.Prelu  # sim_test overrides (Prelu unimplemented in CoreSim)

NC = 8
HEADS = 4
NEG_SLOPE = 0.2
P = 128
B = 128            # dst-block size (slots per bin)
NB = 20            # blocks per core
NPC = B * NB       # node slots per core (2560; 2500 real)
NREAL = 2500       # real nodes per core
NFULL = NC * NPC   # padded global table rows
IN_CH = 128
HC1 = 256
HC2 = 128
OUT_CH = 64


# ----------------------------------------------------------------------------
# device program
# ----------------------------------------------------------------------------

def build_program(NT, dbg=False):
    EB = NT * P               # edges per block (padded)
    IC = EB // 16             # idx cols per block
    nc = bacc.Bacc("TRN2", target_bir_lowering=False, debug=False)

    def inp(name, shape, dt=f32):
        return nc.dram_tensor(name, shape, dt, kind="ExternalInput")

    xT = inp("xT", [IN_CH, NPC], f32r)
    src_idx = inp("src_idx", [P, NB * IC], i16)
    dstrow = inp("dstrow", [NB, EB], bf16)
    dstslot = inp("dstslot", [P, NB * NT], bf16)
    iota_row = inp("iota_row", [P, P], bf16)
    iota_col = inp("iota_col", [P, 1], bf16)

    Wl1T = inp("Wl1T", [IN_CH, HC1], f32r); Wr1T = inp("Wr1T", [IN_CH, HC1], f32r)
    P1T = inp("P1T", [IN_CH, HC1], f32r)
    bl1_bc = inp("bl1_bc", [P, HC1]); br1_bc = inp("br1_bc", [P, HC1])
    pb1m1_bc = inp("pb1m1_bc", [P, HC1]); bias1_bc = inp("bias1_bc", [P, HC1])
    att1_bc = inp("att1_bc", [P, HC1], bf16)

    Wl2T = inp("Wl2T", [HC1, HC2], bf16); Wr2T = inp("Wr2T", [HC1, HC2], bf16)
    P2T = inp("P2T", [HC1, HC2], bf16)
    bl2_bc = inp("bl2_bc", [P, HC2]); br2_bc = inp("br2_bc", [P, HC2])
    pb2m1_bc = inp("pb2m1_bc", [P, HC2]); bias2_bc = inp("bias2_bc", [P, HC2])
    att2_bc = inp("att2_bc", [P, HC2], bf16)

    WfT = inp("WfT", [HC2, OUT_CH], bf16); bf_bc = inp("bf_bc", [P, OUT_CH])

    xl1_own = nc.dram_tensor("xl1_own", [NPC, HC1], bf16)
    xl1_full = nc.dram_tensor("xl1_full", [NFULL, HC1], bf16, addr_space="Shared")
    xl2_own = nc.dram_tensor("xl2_own", [NPC, HC2], bf16)
    xl2_full = nc.dram_tensor("xl2_full", [NFULL, HC2], bf16, addr_space="Shared")
    out_own = nc.dram_tensor("out_own", [NPC, OUT_CH], f32, kind="ExternalOutput")

    with tile.TileContext(nc) as tc:
        with (
            tc.tile_pool(name="const", bufs=1) as cp,
            tc.tile_pool(name="res", bufs=1) as rp,
            tc.tile_pool(name="work", bufs=2) as wp,
            tc.tile_pool(name="gath", bufs=2) as gp,
        ):
            # ---- constants
            ident = cp.tile([P, P], f32)
            make_identity(nc, ident[:])
            identb = cp.tile([P, P], bf16)
            nc.vector.tensor_copy(out=identb[:], in_=ident[:])
            with tc.tile_pool(name="pwarm", bufs=1, space="PSUM") as pw:
                warm = pw.tile([P, P], f32)
                nc.tensor.transpose(out=warm[:], in_=ident[:], identity=ident[:])

            def load_const(t, shape):
                s = cp.tile(shape, t.dtype, tag=f"c_{t.name}")
                nc.sync.dma_start(out=s[:], in_=t[:, :])
                return s

            iota_sb = load_const(iota_row, [P, P])
            iotac_sb = load_const(iota_col, [P, 1])
            zero1_sb = cp.tile([P, HC1], f32)
            nc.gpsimd.memset(zero1_sb[:], 0.0)
            one1_sb = cp.tile([P, HC1], f32)
            nc.gpsimd.memset(one1_sb[:], 1.0)
            w1 = {k: load_const(v, [IN_CH, HC1])
                  for k, v in (("wl", Wl1T), ("wr", Wr1T), ("p", P1T))}
            c1b = {k: load_const(v, [P, HC1]) for k, v in (
                ("bl", bl1_bc), ("br", br1_bc), ("pbm1", pb1m1_bc),
                ("att", att1_bc), ("bias", bias1_bc))}

            def load_w2(t):
                # [HC1, HC2] DRAM -> [128, 2*HC2] SBUF (chunk c at col c*HC2)
                s = cp.tile([P, 2 * HC2], t.dtype, tag=f"c_{t.name}")
                for c in range(2):
                    nc.sync.dma_start(out=s[:, c * HC2:(c + 1) * HC2],
                                      in_=t[c * P:(c + 1) * P, :])
                return s

            w2 = {k: load_w2(v) for k, v in (("wl", Wl2T), ("wr", Wr2T),
                                             ("p", P2T))}
            c2b = {k: load_const(v, [P, HC2]) for k, v in (
                ("bl", bl2_bc), ("br", br2_bc), ("pbm1", pb2m1_bc),
                ("att", att2_bc), ("bias", bias2_bc))}
            wf_sb = load_const(WfT, [HC2, OUT_CH])
            bf_sb = load_const(bf_bc, [P, OUT_CH])

            xT_sb = rp.tile([IN_CH, NPC], f32r)
            nc.sync.dma_start(out=xT_sb[:], in_=xT[:, :])
            sidx_sb = rp.tile([P, NB * IC], i16)
            nc.sync.dma_start(out=sidx_sb[:], in_=src_idx[:, :])
            dsl_sb = rp.tile([P, NB * NT], bf16)
            nc.sync.dma_start(out=dsl_sb[:], in_=dstslot[:, :])

            # ---- persistent per-layer node tensors
            xl1_sb = rp.tile([P, NB * HC1], bf16)
            xr1_sb = rp.tile([P, NB * HC1], bf16)
            id1_sb = rp.tile([P, NB * HC1], bf16)
            em1_sb = rp.tile([P, NB * HEADS], f32)
            h_sb = rp.tile([P, NB * HC1], bf16)
            hT_sb = rp.tile([P, NB * HC1], bf16)     # h transposed, 2 chunks/blk
            xl2_sb = rp.tile([P, NB * HC2], bf16)
            xr2_sb = rp.tile([P, NB * HC2], bf16)
            id2_sb = rp.tile([P, NB * HC2], bf16)
            em2_sb = rp.tile([P, NB * HEADS], f32)
            h2_sb = rp.tile([P, NB * HC2], bf16)

            # ---- N1a: xl for all blocks, then AllGather
            with tc.tile_pool(name="pn1a", bufs=2, space="PSUM") as ps:
                for b in range(NB):
                    pxl = ps.tile([P, HC1], f32, tag="pxl")
                    nc.tensor.matmul(out=pxl[:], lhsT=xT_sb[:, b * B:(b + 1) * B],
                                     rhs=w1["wl"][:], start=True, stop=True)
                    nc.vector.tensor_add(out=xl1_sb[:, b * HC1:(b + 1) * HC1],
                                         in0=pxl[:], in1=c1b["bl"][:])
            nc.sync.dma_start(
                out=xl1_own.ap().rearrange("(b p) c -> p b c", p=B),
                in_=xl1_sb[:].rearrange("p (b c) -> p b c", b=NB))
            nc.gpsimd.collective_compute(
                "AllGather", ALU.bypass,
                replica_groups=[list(range(NC))],
                ins=[xl1_own.ap().opt()], outs=[xl1_full.ap().opt()])

            # ---- N1c: xr, id, m (overlaps the AllGather)
            with tc.tile_pool(name="pn1c", bufs=2, space="PSUM") as ps:
                for b in range(NB):
                    pxr = ps.tile([P, HC1], f32, tag="pxr")
                    pid = ps.tile([P, HC1], f32, tag="pid")
                    lhsT = xT_sb[:, b * B:(b + 1) * B]
                    nc.tensor.matmul(out=pxr[:], lhsT=lhsT, rhs=w1["wr"][:],
                                     start=True, stop=True)
                    nc.tensor.matmul(out=pid[:], lhsT=lhsT, rhs=w1["p"][:],
                                     start=True, stop=True)
                    xr_t = xr1_sb[:, b * HC1:(b + 1) * HC1]
                    nc.vector.tensor_add(out=xr_t, in0=pxr[:], in1=c1b["br"][:])
                    nc.vector.tensor_add(out=id1_sb[:, b * HC1:(b + 1) * HC1],
                                         in0=pid[:], in1=c1b["pbm1"][:])
                    t0 = wp.tile([P, HC1], bf16, tag="t0")
                    nc.vector.tensor_add(out=t0[:],
                                         in0=xl1_sb[:, b * HC1:(b + 1) * HC1],
                                         in1=xr_t)
                    nc.scalar.activation(t0[:], t0[:], PRELU, alpha=NEG_SLOPE)
                    nc.vector.tensor_mul(out=t0[:], in0=t0[:], in1=c1b["att"][:])
                    nc.vector.reduce_sum(
                        out=em1_sb[:, b * HEADS:(b + 1) * HEADS],
                        in_=t0[:].rearrange("p (h c) -> p h c", h=HEADS),
                        axis=mybir.AxisListType.X)
            nc.scalar.activation(em1_sb[:], em1_sb[:], AF.Exp)

            # ---- generic edge phase
            # tt = xl_src + xr_dst built on the PE: per tile, ohT @ xr_block
            # accumulated with I @ xlg in PSUM; Prelu reads PSUM on Scalar.
            def edge_phase(b, full_tab, xr_sb, xl_sb, id_sb, em_sb, hout_sb,
                           hc, consts, psa, pst):
                ic0 = b * IC
                hf = NT // 2          # dma_gather wedges above 1024 idxs
                eh, ich = hf * P, IC // 2
                xlg = gp.tile([P, NT, hc], bf16, tag="xlg")
                for i in range(2):
                    nc.gpsimd.dma_gather(
                        xlg[:, i * hf:(i + 1) * hf, :], full_tab[:, :],
                        sidx_sb[:, ic0 + i * ich:ic0 + (i + 1) * ich],
                        eh, eh, hc)
                dr = gp.tile([P, EB], bf16, tag="dr")
                nc.sync.dma_start(
                    out=dr[:], in_=dstrow.ap()[b:b + 1, :].partition_broadcast(P))
                ohT = wp.tile([P, NT, P], bf16, tag="ohT")
                nc.vector.tensor_tensor(
                    out=ohT[:].rearrange("p t q -> p (t q)"),
                    in0=iotac_sb[:].to_broadcast([P, EB]),
                    in1=dr[:], op=ALU.is_equal)
                oh = wp.tile([P, NT, P], bf16, tag="oh")
                nc.vector.tensor_tensor(
                    out=oh[:],
                    in0=dsl_sb[:, b * NT:(b + 1) * NT].unsqueeze(2)
                        .to_broadcast([P, NT, P]),
                    in1=iota_sb[:].unsqueeze(1).to_broadcast([P, NT, P]),
                    op=ALU.is_equal)
                xrb = xr_sb[:, b * hc:(b + 1) * hc]
                tb = wp.tile([P, NT, hc], bf16, tag="tb")
                for j in range(NT // 2):
                    ttp = pst.tile([P, 2, hc], f32, tag="ttp")
                    for i in range(2):
                        k = 2 * j + i
                        nc.tensor.matmul(out=ttp[:, i, :], lhsT=ohT[:, k, :],
                                         rhs=xrb, start=True, stop=False)
                        nc.tensor.matmul(out=ttp[:, i, :], lhsT=identb[:],
                                         rhs=xlg[:, k, :], start=False, stop=True)
                    nc.scalar.activation(
                        tb[:, 2 * j:2 * j + 2, :].rearrange("p t e -> p (t e)"),
                        ttp[:].rearrange("p t e -> p (t e)"),
                        PRELU, alpha=NEG_SLOPE)
                nc.vector.tensor_mul(
                    out=tb[:], in0=tb[:],
                    in1=consts["att"][:].unsqueeze(1).to_broadcast([P, NT, hc]))
                lgf = wp.tile([P, NT * HEADS], f32, tag="lgf")
                nc.vector.reduce_sum(
                    out=lgf[:].rearrange("p (t h) -> p t h", t=NT),
                    in_=tb[:].rearrange("p t (h c) -> p t h c", h=HEADS),
                    axis=mybir.AxisListType.X)
                pexp = wp.tile([P, NT * HEADS], bf16, tag="pexp")
                nc.scalar.activation(pexp[:], lgf[:], AF.Exp)
                wptn = wp.tile([P, NT, hc], bf16, tag="wptn")
                nc.vector.tensor_tensor(
                    out=wptn[:].rearrange("p t (h c) -> p t h c", h=HEADS),
                    in0=xlg[:].rearrange("p t (h c) -> p t h c", h=HEADS),
                    in1=pexp[:].rearrange("p (t h) -> p t h", t=NT).unsqueeze(3)
                        .to_broadcast([P, NT, HEADS, hc // HEADS]),
                    op=ALU.mult)
                seg = psa.tile([P, hc], f32, tag="seg")
                segE = psa.tile([P, HEADS], f32, tag="segE")
                pex3 = pexp[:].rearrange("p (t h) -> p t h", t=NT)
                for k in range(NT):
                    nc.tensor.matmul(out=seg[:], lhsT=oh[:, k, :],
                                     rhs=wptn[:, k, :],
                                     start=(k == 0), stop=(k == NT - 1))
                for k in range(NT):
                    nc.tensor.matmul(out=segE[:], lhsT=oh[:, k, :],
                                     rhs=pex3[:, k, :],
                                     start=(k == 0), stop=(k == NT - 1))
                # finalize block b
                e4 = em_sb[:, b * HEADS:(b + 1) * HEADS]
                den = wp.tile([P, HEADS], f32, tag="den")
                nc.vector.tensor_add(out=den[:], in0=segE[:], in1=e4)
                rec = wp.tile([P, HEADS], f32, tag="rec")
                nc.vector.reciprocal(out=rec[:], in_=den[:])
                num = wp.tile([P, hc], f32, tag="num")
                nc.vector.tensor_tensor(
                    out=num[:].rearrange("p (h c) -> p h c", h=HEADS),
                    in0=xl_sb[:, b * hc:(b + 1) * hc]
                        .rearrange("p (h c) -> p h c", h=HEADS),
                    in1=e4.unsqueeze(2).to_broadcast([P, HEADS, hc // HEADS]),
                    op=ALU.mult)
                nc.vector.tensor_add(out=num[:], in0=num[:], in1=seg[:])
                nc.vector.tensor_tensor(
                    out=num[:].rearrange("p (h c) -> p h c", h=HEADS),
                    in0=num[:].rearrange("p (h c) -> p h c", h=HEADS),
                    in1=rec[:].unsqueeze(2).to_broadcast([P, HEADS, hc // HEADS]),
                    op=ALU.mult)
                nc.vector.tensor_add(out=num[:], in0=num[:], in1=consts["bias"][:])
                eu = wp.tile([P, hc], f32, tag="eu")
                nc.scalar.activation(eu[:], num[:], AF.Exp)
                nc.vector.tensor_tensor(out=eu[:], in0=eu[:],
                                        in1=one1_sb[:, :hc], op=ALU.min)
                nc.vector.tensor_tensor(out=num[:], in0=num[:],
                                        in1=zero1_sb[:, :hc], op=ALU.max)
                nc.vector.tensor_add(out=num[:], in0=num[:], in1=eu[:])
                nc.vector.tensor_add(out=hout_sb[:, b * hc:(b + 1) * hc],
                                     in0=num[:], in1=id_sb[:, b * hc:(b + 1) * hc])

            with (
                tc.tile_pool(name="pe1", bufs=2, space="PSUM") as psa,
                tc.tile_pool(name="pt1", bufs=4, space="PSUM") as pst,
            ):
                for b in range(NB):
                    edge_phase(b, xl1_full, xr1_sb, xl1_sb, id1_sb, em1_sb,
                               h_sb, HC1, c1b, psa, pst)

            # ---- N2a: transpose h; N2b: xl2 + AllGather; N2c: xr2/id2/m2
            with tc.tile_pool(name="pn2a", bufs=2, space="PSUM") as ps:
                for b in range(NB):
                    for c in range(2):
                        tps = ps.tile([P, P], bf16, tag="hT_ps")
                        nc.tensor.transpose(
                            out=tps[:],
                            in_=h_sb[:, b * HC1 + c * P:b * HC1 + (c + 1) * P],
                            identity=identb[:])
                        nc.scalar.copy(
                            out=hT_sb[:, b * HC1 + c * P:b * HC1 + (c + 1) * P],
                            in_=tps[:])
            with tc.tile_pool(name="pn2b", bufs=2, space="PSUM") as ps:
                for b in range(NB):
                    pxl = ps.tile([P, HC2], f32, tag="p2xl")
                    for c in range(2):
                        nc.tensor.matmul(
                            out=pxl[:],
                            lhsT=hT_sb[:, b * HC1 + c * P:b * HC1 + (c + 1) * P],
                            rhs=w2["wl"][:, c * HC2:(c + 1) * HC2],
                            start=(c == 0), stop=(c == 1))
                    nc.vector.tensor_add(out=xl2_sb[:, b * HC2:(b + 1) * HC2],
                                         in0=pxl[:], in1=c2b["bl"][:])
            nc.sync.dma_start(
                out=xl2_own.ap().rearrange("(b p) c -> p b c", p=B),
                in_=xl2_sb[:].rearrange("p (b c) -> p b c", b=NB))
            nc.gpsimd.collective_compute(
                "AllGather", ALU.bypass,
                replica_groups=[list(range(NC))],
                ins=[xl2_own.ap().opt()], outs=[xl2_full.ap().opt()])
            with tc.tile_pool(name="pn2c", bufs=2, space="PSUM") as ps:
                for b in range(NB):
                    pxr = ps.tile([P, HC2], f32, tag="p2xr")
                    pid = ps.tile([P, HC2], f32, tag="p2id")
                    for c in range(2):
                        lhsT = hT_sb[:, b * HC1 + c * P:b * HC1 + (c + 1) * P]
                        st, sp = (c == 0), (c == 1)
                        cs = slice(c * HC2, (c + 1) * HC2)
                        nc.tensor.matmul(out=pxr[:], lhsT=lhsT,
                                         rhs=w2["wr"][:, cs], start=st, stop=sp)
                        nc.tensor.matmul(out=pid[:], lhsT=lhsT,
                                         rhs=w2["p"][:, cs], start=st, stop=sp)
                    xr_t = xr2_sb[:, b * HC2:(b + 1) * HC2]
                    nc.vector.tensor_add(out=xr_t, in0=pxr[:], in1=c2b["br"][:])
                    nc.vector.tensor_add(out=id2_sb[:, b * HC2:(b + 1) * HC2],
                                         in0=pid[:], in1=c2b["pbm1"][:])
                    t0 = wp.tile([P, HC2], bf16, tag="t02")
                    nc.vector.tensor_add(out=t0[:],
                                         in0=xl2_sb[:, b * HC2:(b + 1) * HC2],
                                         in1=xr_t)
                    nc.scalar.activation(t0[:], t0[:], PRELU, alpha=NEG_SLOPE)
                    nc.vector.tensor_mul(out=t0[:], in0=t0[:], in1=c2b["att"][:])
                    nc.vector.reduce_sum(
                        out=em2_sb[:, b * HEADS:(b + 1) * HEADS],
                        in_=t0[:].rearrange("p (h c) -> p h c", h=HEADS),
                        axis=mybir.AxisListType.X)
            nc.scalar.activation(em2_sb[:], em2_sb[:], AF.Exp)

            # ---- layer-2 edge phase + FC
            with (
                tc.tile_pool(name="pe2", bufs=2, space="PSUM") as psa,
                tc.tile_pool(name="pt2", bufs=4, space="PSUM") as pst,
            ):
                for b in range(NB):
                    edge_phase(b, xl2_full, xr2_sb, xl2_sb, id2_sb, em2_sb,
                               h2_sb, HC2, c2b, psa, pst)
            with tc.tile_pool(name="pfc", bufs=2, space="PSUM") as ps:
                for b in range(NB):
                    tps = ps.tile([P, P], bf16, tag="fc_ps")
                    nc.tensor.transpose(
                        out=tps[:], in_=h2_sb[:, b * HC2:(b + 1) * HC2],
                        identity=identb[:])
                    h2T = wp.tile([P, P], bf16, tag="fcT")
                    nc.scalar.copy(out=h2T[:], in_=tps[:])
                    pf = ps.tile([P, OUT_CH], f32, tag="fc_out")
                    nc.tensor.matmul(out=pf[:], lhsT=h2T[:], rhs=wf_sb[:],
                                     start=True, stop=True)
                    ob = wp.tile([P, OUT_CH], f32, tag="fc_ob")
                    nc.vector.tensor_add(out=ob[:], in0=pf[:], in1=bf_sb[:])
                    nc.sync.dma_start(out=out_own.ap()[b * B:(b + 1) * B, :],
                                      in_=ob[:])
    nc.compile()
    return nc


# ----------------------------------------------------------------------------
# host-side sharding / input prep
# ----------------------------------------------------------------------------

def balanced_assignment(dst):
    """Snake-deal nodes (by in-degree desc) into NC*NB bins of NREAL/NB..."""
    N = NC * NREAL
    nbins = NC * NB
    rounds = N // nbins  # 125
    deg = np.bincount(dst, minlength=N)
    order = np.argsort(-deg, kind="stable")
    bin_of = np.empty(N, np.int32)
    slot_of = np.empty(N, np.int32)
    cols = np.arange(nbins)
    for r in range(rounds):
        nodes = order[r * nbins:(r + 1) * nbins]
        c = cols if r % 2 == 0 else cols[::-1]
        bin_of[nodes] = c
        slot_of[nodes] = r
    return bin_of, slot_of


def wrap_idx(vals, EB):
    """[EB] int -> [128, EB//16] int16 wrapped in 16 partitions, replicated."""
    w = np.zeros((16, EB // 16), np.int16)
    w[np.arange(EB) % 16, np.arange(EB) // 16] = vals.astype(np.int16)
    return np.tile(w, (8, 1))


def prep_inputs(x, edge_index, weights):
    src = np.asarray(edge_index[0], dtype=np.int64)
    dst = np.asarray(edge_index[1], dtype=np.int64)
    E = src.shape[0]

    bin_of, slot_of = balanced_assignment(dst)
    core_of = bin_of // NB
    block_of = bin_of % NB
    gid = core_of * NPC + block_of * B + slot_of  # row in full tables

    ebin = bin_of[dst]
    counts = np.bincount(ebin, minlength=NC * NB)
    NT = max(1, int(np.ceil(counts.max() / P)))
    EB = NT * P
    IC = EB // 16

    eorder = np.argsort(ebin, kind="stable")
    offs = np.zeros(NC * NB + 1, np.int64)
    np.cumsum(counts, out=offs[1:])

    src_gid = gid[src]
    dst_slot = slot_of[dst]

    in_maps = []
    for c in range(NC):
        sarr = np.zeros((P, NB * IC), np.int16)
        drow = np.full((NB, EB), -1.0, np.float32)
        slarr = np.full((P, NB * NT), -1.0, np.float32)
        for b in range(NB):
            bi = c * NB + b
            eb = eorder[offs[bi]:offs[bi + 1]]
            n = len(eb)
            sv = np.zeros(EB, np.int64); sv[:n] = src_gid[eb]
            sarr[:, b * IC:(b + 1) * IC] = wrap_idx(sv, EB)
            drow[b, :n] = dst_slot[eb]
            j = np.arange(n)
            slarr[j % P, b * NT + j // P] = dst_slot[eb]
        in_maps.append({"src_idx": sarr, "dstrow": drow.astype(NPBF),
                        "dstslot": slarr.astype(NPBF)})

    def bc(v):
        return np.tile(np.asarray(v, np.float32)[None, :], (P, 1))

    consts = {
        "iota_row": np.tile(np.arange(P), (P, 1)).astype(NPBF),
        "iota_col": np.arange(P).reshape(P, 1).astype(NPBF),
        "Wl1T": np.ascontiguousarray(weights["Wl1"].T.astype(np.float32)),
        "Wr1T": np.ascontiguousarray(weights["Wr1"].T.astype(np.float32)),
        "P1T": np.ascontiguousarray(weights["P1"].T.astype(np.float32)),
        "bl1_bc": bc(weights["bl1"]), "br1_bc": bc(weights["br1"]),
        "pb1m1_bc": bc(weights["pb1"] - 1.0), "bias1_bc": bc(weights["bias1"]),
        "att1_bc": bc(weights["att1"].reshape(-1)).astype(NPBF),
        "Wl2T": np.ascontiguousarray(weights["Wl2"].T.astype(NPBF)),
        "Wr2T": np.ascontiguousarray(weights["Wr2"].T.astype(NPBF)),
        "P2T": np.ascontiguousarray(weights["P2"].T.astype(NPBF)),
        "bl2_bc": bc(weights["bl2"]), "br2_bc": bc(weights["br2"]),
        "pb2m1_bc": bc(weights["pb2"] - 1.0), "bias2_bc": bc(weights["bias2"]),
        "att2_bc": bc(weights["att2"].reshape(-1)).astype(NPBF),
        "WfT": np.ascontiguousarray(weights["Wf"].T.astype(NPBF)),
        "bf_bc": bc(weights["bf"]),
    }

    # permuted x, transposed: col (block*B+slot) = x[node]
    xf = np.asarray(x, np.float32)
    for c in range(NC):
        xp = np.zeros((NPC, IN_CH), np.float32)
        m = core_of == c
        xp[block_of[m] * B + slot_of[m]] = xf[m]
        im = in_maps[c]
        im["xT"] = np.ascontiguousarray(xp.T)
        im.update(consts)
    return in_maps, NT, (core_of, block_of, slot_of)


_CACHE = {}


def kernel(x, edge_index, Wl1, bl1, Wr1, br1, att1, bias1, P1, pb1,
           Wl2, bl2, Wr2, br2, att2, bias2, P2, pb2, Wf, bf):
    x = np.asarray(x)
    weights = dict(Wl1=np.asarray(Wl1), bl1=np.asarray(bl1),
                   Wr1=np.asarray(Wr1), br1=np.asarray(br1),
                   att1=np.asarray(att1), bias1=np.asarray(bias1),
                   P1=np.asarray(P1), pb1=np.asarray(pb1),
                   Wl2=np.asarray(Wl2), bl2=np.asarray(bl2),
                   Wr2=np.asarray(Wr2), br2=np.asarray(br2),
                   att2=np.asarray(att2), bias2=np.asarray(bias2),
                   P2=np.asarray(P2), pb2=np.asarray(pb2),
                   Wf=np.asarray(Wf), bf=np.asarray(bf))
    assert x.shape[0] == NC * NREAL, "hardcoded for the BrainGAT problem size"
    in_maps, NT, (core_of, block_of, slot_of) = prep_inputs(
        x, np.asarray(edge_index), weights)
    if NT not in _CACHE:
        _CACHE[NT] = build_program(NT)
    nc = _CACHE[NT]
    res = run_bass_kernel_spmd(nc, in_maps, list(range(NC)))
    full = np.concatenate([res.results[c]["out_own"] for c in range(NC)], 0)
    rows = core_of * NPC + block_of * B + slot_of
    return full[rows].astype(np.float32)
